# revision 2
# baseline (speedup 1.0000x reference)
"""AttentionalPropagation on 8 TRN2 NeuronCores — v3.

Data parallel over batch (B=8 -> one element per core). Math identical to v2
(bf16 matmuls f32 accum, Wm folded into W1, rstd folded into W2, exp without
max-subtraction). v3 changes the plumbing:

  - Inputs land partition-major and contiguous (one DMA packet per partition)
    and the dma_start instructions are spread across four engine queues so
    descriptor generation doesn't serialize the prologue.
  - The whole kernel is a sequence of 8 ACT-paced "windows" W(j, p) of 11
    score/exp groups (3x512 psum slots -> 1536-elem exp ACTs, 6-bank
    ping-pong). Aux tensor work (den duos, msg pairs, h1, QKV/vT projections)
    is emitted as generator segments, one per group, sized ~1.2us so the PE
    queue never buries the score matmuls the exp stream is waiting on.
  - scores: row-tiled K=64 pairs (heads on partitions 0-63/64-127 run
    concurrently); msg: col-tiled M=64 pairs; den: col-tiled M=1 duos.
  - Aux psum: 2 banks ping-pong (den/msg/h1/qk/vT/out chains are compact).

Window layout (den/msg of (j,p) consumed one window later):
  W(0,p0): scores + [k-p1, q-p1-j0, vT 0..9]
  W(0,p1): scores + [vT 10..15, den(0,p0), msg(0,p0), q j1..3]
  W(j,p0): scores + [den(j-1,p1), msg(j-1,p1), h1(j-1) o0,o1]
  W(j,p1): scores + [den(j,p0),   msg(j,p0),   h1(j-1) o2,o3]
  tail:    den(3,p1), msg(3,p1), h1(3), InstanceNorm, relu, W2, out
"""

import os
import sys

for _p in ("/opt/trn_rl_repo",):
    if _p not in sys.path:
        sys.path.insert(0, _p)

import numpy as np
import ml_dtypes

import concourse.bass as bass
import concourse.mybir as mybir
from concourse import bacc
from concourse import library_config
from concourse.bass import ts
from concourse.tile import TileContext
from concourse.bass_utils import run_bass_kernel_spmd

F32 = mybir.dt.float32
F16 = mybir.dt.float16
BF16 = mybir.dt.bfloat16
AF = mybir.ActivationFunctionType
ALU = mybir.AluOpType

B, D, N, M, H, DH = 8, 256, 2048, 2048, 4, 64
EPS = 1e-5
NCH = 4
CHUNK = 512
NMT = M // 128           # 16 m-tiles
PSLOTS = NMT * 2         # 32 slots per (j, p)
NGRP = (PSLOTS + 2) // 3  # 11 groups (10x3 + 1x2)


def _build():
    nc = bacc.Bacc("TRN2", target_bir_lowering=False, debug=False, num_devices=8)

    x_ds = [nc.dram_tensor(f"x{j}", [128, 2, CHUNK], BF16,
                           kind="ExternalInput").ap()
            for j in range(4)]
    s_ds = [nc.dram_tensor(f"src{j}", [128, 2, CHUNK], BF16,
                           kind="ExternalInput").ap()
            for j in range(4)]
    wq_d = nc.dram_tensor("wqT", [128, 2, D], BF16, kind="ExternalInput").ap()
    wk_d = nc.dram_tensor("wkT", [128, 2, D], BF16, kind="ExternalInput").ap()
    wv_d = nc.dram_tensor("wvT", [128, 2, D], BF16, kind="ExternalInput").ap()
    w1_d = nc.dram_tensor("w1T", [128, 4, 2 * D], BF16, kind="ExternalInput").ap()
    w2_d = nc.dram_tensor("w2T", [128, 4, D], BF16, kind="ExternalInput").ap()
    bias_d = nc.dram_tensor("bias", [128, 2, 3], F32, kind="ExternalInput").ap()
    bv_d = nc.dram_tensor("bv", [1, D], BF16, kind="ExternalInput").ap()
    out_d = nc.dram_tensor("out", [128, 4, 2, CHUNK], F16,
                           kind="ExternalOutput").ap()

    with TileContext(nc) as tc:
        nc.gpsimd.load_library(library_config.attn)
        with (
            tc.tile_pool(name="const", bufs=1) as const,
            tc.tile_pool(name="data", bufs=1) as data,
            tc.tile_pool(name="reuse", bufs=2) as reuse,
            tc.tile_pool(name="exps", bufs=3) as exps,
            tc.tile_pool(name="small", bufs=2) as small,
            tc.tile_pool(name="rbcs", bufs=2) as rbcs,
            tc.tile_pool(name="msgn", bufs=4) as msgn,
            tc.tile_pool(name="gate", bufs=1) as gatep,
            tc.tile_pool(name="ps_sc", bufs=2, space="PSUM") as ps_sc,
            tc.tile_pool(name="ps_aux", bufs=2, space="PSUM") as ps_aux,
        ):
            # ---- priority DMAs (contiguous per partition): the tensors that
            # gate the first K/Q projections ride sync/scalar immediately ----
            s_sb = reuse.tile([128, 2, M], BF16, name="s", tag="big")
            wk_sb = const.tile([128, 2, D], BF16, name="wk")
            x_sb = data.tile([128, 2, N], BF16, name="x")
            wq_sb = const.tile([128, 2, D], BF16, name="wq")
            wv_sb = const.tile([128, 2, D], BF16, name="wv")
            w1_sb = const.tile([128, 4, 2 * D], BF16, name="w1")
            w2_sb = const.tile([128, 4, D], BF16, name="w2")
            bias_sb = const.tile([128, 2, 3], F32, name="bias")
            bv_bc = const.tile([128, D], BF16, name="bvbc")

            nc.sync.dma_start(out=bias_sb[:], in_=bias_d)
            nc.sync.dma_start(out=s_sb[:, :, 0:CHUNK], in_=s_ds[0])
            nc.scalar.dma_start(out=wk_sb[:], in_=wk_d)
            nc.scalar.dma_start(out=wq_sb[:], in_=wq_d)
            nc.scalar.dma_start(out=wv_sb[:], in_=wv_d)
            nc.sync.dma_start(out=x_sb[:, :, 0:CHUNK], in_=x_ds[0])
            for jm in range(1, 4):
                nc.sync.dma_start(out=s_sb[:, :, ts(jm, CHUNK)], in_=s_ds[jm])

            eps_sb = const.tile([128, 1], F32, name="eps")
            nc.vector.memset(eps_sb[:], EPS)
            ones_sb = const.tile([128, 4], BF16, name="ones")
            nc.vector.memset(ones_sb[:], 1.0)
            dummy_sb = const.tile([128, 128], BF16, name="dummy")
            nc.vector.memset(dummy_sb[:], 0.0)
            for _ in range(6):
                wup = ps_aux.tile([128, 512], F32, name="wup", tag="aux")
                nc.tensor.matmul(wup[:, 0:128], dummy_sb[:], dummy_sb[:],
                                 start=True, stop=True)

            # ---- persistent SBUF ----
            q_sb = data.tile([128, 2, N], BF16, name="q")
            k_sb = data.tile([128, 2, M], BF16, name="k")
            vT_sb = [data.tile([128, H, DH], BF16, name=f"vT{t}")
                     for t in range(NMT)]
            h1_sb = data.tile([128, 4, N], BF16, name="h1")
            stats_sb = data.tile([128, 4, NCH, 6], F32, name="stats")

            eS = {}   # (j, p) -> [128, 32, 512] bf16
            mn = {}   # (j, p) -> [128, 512] bf16
            rbc = {}  # (j, h) -> [64, 512] f32

            def eS_view(j, p):
                return eS[(j, p)][:].rearrange("q (mt h) n -> q mt h n", h=2)

            # ---- aux emitters (generators yield ~1.2us segments) ----
            def qk_chunk(dst, w_sb, p, jm, b_col, src_t):
                ps = ps_aux.tile([128, CHUNK], F32, name="qk", tag="aux")
                for c in range(2):
                    nc.tensor.matmul(
                        ps[:],
                        w_sb[:, c, ts(p, 128)],
                        src_t[:, c, ts(jm, CHUNK)],
                        start=(c == 0),
                        stop=(c == 1),
                    )
                nc.vector.tensor_scalar_add(
                    dst[:, p, ts(jm, CHUNK)], ps[:], bias_sb[:, p, b_col : b_col + 1]
                )

            def vT_one(t):
                vp = ps_aux.tile([128, D], F32, name="vps", tag="aux")
                for c in range(2):
                    nc.tensor.matmul(
                        vp[:],
                        s_sb[:, c, ts(t, 128)],
                        wv_sb[:, c, :],
                        start=(c == 0),
                        stop=(c == 1),
                    )
                nc.vector.tensor_add(
                    vT_sb[t][:],
                    vp[:].rearrange("p (h d) -> p h d", h=H),
                    bv_bc[:].rearrange("p (h d) -> p h d", h=H),
                )

            def den_gen(j, p):
                dp = ps_aux.tile([128, CHUNK], F32, name="denps", tag="aux")
                v = eS_view(j, p)
                for mt in range(NMT):
                    if mt and mt % 4 == 0:
                        yield
                    for h2 in range(2):
                        h = 2 * p + h2
                        nc.tensor.matmul(
                            dp[32 * h : 32 * h + 1, :],
                            ones_sb[:, h : h + 1],
                            v[:, mt, h2, :],
                            start=(mt == 0),
                            stop=(mt == NMT - 1),
                            tile_position=(0, 32 * h),
                        )
                dens, rdens = [], []
                for h2 in range(2):
                    h = 2 * p + h2
                    den = small.tile([1, CHUNK], F32, name="den", tag="den")
                    nc.vector.tensor_copy(den[:], dp[32 * h : 32 * h + 1, :])
                    dens.append(den)
                for h2 in range(2):
                    rden = small.tile([1, CHUNK], F32, name="rden", tag="rden")
                    nc.vector.reciprocal_approx_fast(rden[:], dens[h2][:])
                    rdens.append(rden)
                for h2 in range(2):
                    rb = rbcs.tile([DH, CHUNK], F32, name="rbc", tag="rbc")
                    nc.gpsimd.partition_broadcast(rb[:], rdens[h2][:])
                    rbc[(j, 2 * p + h2)] = rb

            def msg_gen(j, p):
                mp = ps_aux.tile([128, CHUNK], F32, name="msgps", tag="aux")
                v = eS_view(j, p)
                for mt in range(NMT):
                    if mt and mt % 4 == 0:
                        yield
                    for h2 in range(2):
                        nc.tensor.matmul(
                            mp[ts(h2, DH), :],
                            vT_sb[mt][:, 2 * p + h2, :],
                            v[:, mt, h2, :],
                            start=(mt == 0),
                            stop=(mt == NMT - 1),
                        )
                mnp = msgn.tile([128, CHUNK], BF16, name="mn", tag="mn")
                for h2 in range(2):
                    nc.vector.tensor_mul(
                        mnp[ts(h2, DH), :],
                        mp[ts(h2, DH), :],
                        rbc.pop((j, 2 * p + h2))[:],
                    )
                mn[(j, p)] = mnp

            def h1_gen(j, olist):
                for o in olist:
                    hp = ps_aux.tile([128, CHUNK], F32, name="h1ps", tag="aux")
                    for c in range(4):
                        rhs = (
                            x_sb[:, c, ts(j, CHUNK)] if c < 2 else mn[(j, c - 2)][:]
                        )
                        nc.tensor.matmul(
                            hp[:],
                            w1_sb[:, c, ts(o, 128)],
                            rhs,
                            start=(c == 0),
                            stop=(c == 3),
                        )
                    nc.vector.tensor_copy(h1_sb[:, o, ts(j, CHUNK)], hp[:])
                    nc.vector.bn_stats(
                        stats_sb[:, o, j, :], h1_sb[:, o, ts(j, CHUNK)]
                    )
                    if o != olist[-1]:
                        yield

            def list_gen(thunks, per_seg):
                for i, t in enumerate(thunks):
                    if i and i % per_seg == 0:
                        yield
                    t()

            class Trail:
                """den duo + msg pair chains of (j, p) emitted mt-by-mt,
                lagging the exp stream by 2 ACT groups (used only for the
                final window so its softmax consumers finish with the exps).
                """

                def __init__(self, j, p):
                    self.j, self.p = j, p
                    self.mt = 0
                    self.dp = ps_aux.tile([128, CHUNK], F32, name="denps",
                                          tag="aux")
                    self.mp = ps_aux.tile([128, CHUNK], F32, name="msgps",
                                          tag="aux")
                    self.v = eS_view(j, p)

                def advance(self, mt_lim):
                    p = self.p
                    while self.mt <= min(mt_lim, NMT - 1):
                        mt = self.mt
                        for h2 in range(2):
                            h = 2 * p + h2
                            nc.tensor.matmul(
                                self.dp[32 * h : 32 * h + 1, :],
                                ones_sb[:, h : h + 1],
                                self.v[:, mt, h2, :],
                                start=(mt == 0),
                                stop=(mt == NMT - 1),
                                tile_position=(0, 32 * h),
                            )
                        for h2 in range(2):
                            nc.tensor.matmul(
                                self.mp[ts(h2, DH), :],
                                vT_sb[mt][:, 2 * p + h2, :],
                                self.v[:, mt, h2, :],
                                start=(mt == 0),
                                stop=(mt == NMT - 1),
                            )
                        self.mt += 1

                def finish(self):
                    self.advance(NMT - 1)
                    j, p = self.j, self.p
                    dens, rdens = [], []
                    for h2 in range(2):
                        h = 2 * p + h2
                        den = small.tile([1, CHUNK], F32, name="den", tag="den")
                        nc.vector.tensor_copy(
                            den[:], self.dp[32 * h : 32 * h + 1, :]
                        )
                        dens.append(den)
                    for h2 in range(2):
                        rden = small.tile([1, CHUNK], F32, name="rden",
                                          tag="rden")
                        nc.vector.reciprocal_approx_fast(rden[:], dens[h2][:])
                        rdens.append(rden)
                    for h2 in range(2):
                        rb = rbcs.tile([DH, CHUNK], F32, name="rbc", tag="rbc")
                        nc.gpsimd.partition_broadcast(rb[:], rdens[h2][:])
                        rbc[(j, 2 * p + h2)] = rb
                    mnp = msgn.tile([128, CHUNK], BF16, name="mn", tag="mn")
                    for h2 in range(2):
                        nc.vector.tensor_mul(
                            mnp[ts(h2, DH), :],
                            self.mp[ts(h2, DH), :],
                            rbc.pop((j, 2 * p + h2))[:],
                        )
                    mn[(j, p)] = mnp

            # ---- window: 11 ACT-paced score groups + one aux segment each ----
            def window(j, p, gens, trail=False):
                eS[(j, p)] = exps.tile(
                    [128, PSLOTS, CHUNK], BF16, name="eS", tag="eS"
                )
                gq = list(gens)
                tr = None
                for g in range(NGRP):
                    lo = 3 * g
                    nu = min(3, PSLOTS - lo)
                    sc = ps_sc.tile([128, 3, CHUNK], F32, name="sc", tag="sc")
                    for u in range(nu):
                        mt, h2 = divmod(lo + u, 2)
                        nc.tensor.matmul(
                            sc[:, u, :],
                            k_sb[ts(h2, DH), p, ts(mt, 128)],
                            q_sb[ts(h2, DH), p, ts(j, CHUNK)],
                            start=True,
                            stop=True,
                        )
                    nc.scalar.activation(
                        eS[(j, p)][:, lo : lo + nu, :],
                        sc[:, 0:nu, :],
                        AF.Exp,
                        scale=1.0 / 8.0,
                    )
                    if gq:
                        try:
                            next(gq[0])
                        except StopIteration:
                            gq.pop(0)
                    elif trail and g >= 2:
                        if tr is None:
                            tr = Trail(j, p)
                        tr.advance((3 * g - 5) // 2)
                while gq:
                    try:
                        next(gq[0])
                    except StopIteration:
                        gq.pop(0)
                return tr

            # ---- schedule ----
            # prologue: K proj chunk 0 gates scores(0, p0) group 0
            qk_chunk(k_sb, wk_sb, 0, 0, 1, s_sb)
            # deferred DMAs: WAW pre-touch via the gate value keeps the
            # scheduler from hoisting them into the priority transfer window
            gate_sb = gatep.tile([1, 4], BF16, name="gate")
            nc.gpsimd.tensor_copy(gate_sb[:], k_sb[0:1, 0, 0:4])
            deferred = [
                (x_sb[0:1, 0, CHUNK : CHUNK + 1],
                 x_sb[:, :, ts(1, CHUNK)], x_ds[1]),
                (x_sb[0:1, 0, 2 * CHUNK : 2 * CHUNK + 1],
                 x_sb[:, :, ts(2, CHUNK)], x_ds[2]),
                (x_sb[0:1, 0, 3 * CHUNK : 3 * CHUNK + 1],
                 x_sb[:, :, ts(3, CHUNK)], x_ds[3]),
                (w1_sb[0:1, 0, 0:1], w1_sb[:], w1_d),
                (w2_sb[0:1, 0, 0:1], w2_sb[:], w2_d),
            ]
            for touch, dst, src in deferred:
                nc.vector.tensor_copy(touch, gate_sb[0:1, 0:1])
            for touch, dst, src in deferred:
                nc.gpsimd.dma_start(out=dst, in_=src)
            bv_src = bass.AP(
                tensor=bv_d.tensor, offset=bv_d.offset, ap=[[0, 128]] + bv_d.ap[1:]
            )
            nc.vector.tensor_copy(bv_bc[0:1, 0:1], gate_sb[0:1, 0:1])
            nc.gpsimd.dma_start(out=bv_bc[:], in_=bv_src)
            qk_chunk(q_sb, wq_sb, 0, 0, 0, x_sb)
            for jm in range(1, 4):
                qk_chunk(k_sb, wk_sb, 0, jm, 1, s_sb)

            window(0, 0, [
                list_gen([lambda jm=jm: qk_chunk(k_sb, wk_sb, 1, jm, 1, s_sb)
                          for jm in range(4)]
                         + [lambda: qk_chunk(q_sb, wq_sb, 1, 0, 0, x_sb)], 2),
                list_gen([lambda t=t: vT_one(t) for t in range(10)], 3),
            ])
            window(0, 1, [
                list_gen([lambda t=t: vT_one(t) for t in range(10, 16)], 3),
                den_gen(0, 0),
                msg_gen(0, 0),
                list_gen([lambda p=p, jm=jm: qk_chunk(q_sb, wq_sb, p, jm, 0, x_sb)
                          for jm in range(1, 4) for p in range(2)], 2),
            ])
            for j in range(1, NCH - 1):
                window(j, 0, [
                    den_gen(j - 1, 1),
                    msg_gen(j - 1, 1),
                    h1_gen(j - 1, [0, 1]),
                ])
                window(j, 1, [
                    den_gen(j, 0),
                    msg_gen(j, 0),
                    h1_gen(j - 1, [2, 3]),
                ])
            # last chunk: W(3,p1) trails its own den/msg; h1(2) o2/o3 move to
            # the tail where the PE would otherwise idle during the mn chain
            window(3, 0, [
                den_gen(2, 1),
                msg_gen(2, 1),
                h1_gen(2, [0, 1]),
            ])
            tr = window(3, 1, [
                den_gen(3, 0),
                msg_gen(3, 0),
            ], trail=True)

            jL = NCH - 1
            if tr is None:
                tr = Trail(jL, 1)
            tr.finish()
            # PE work whose deps are long ready fills the mn-chain latency
            for _ in h1_gen(2, [2, 3]):
                pass
            for _ in range(10):
                wup2 = ps_sc.tile([128, 3, CHUNK], F32, name="wup2", tag="sc")
                nc.tensor.matmul(wup2[:, 0, 0:128], dummy_sb[:], dummy_sb[:],
                                 start=True, stop=True)
            for _ in h1_gen(jL, [0, 1, 2, 3]):
                pass
            for _ in range(26):
                wup3 = ps_sc.tile([128, 3, CHUNK], F32, name="wup3", tag="sc")
                nc.tensor.matmul(wup3[:, 0, 0:128], dummy_sb[:], dummy_sb[:],
                                 start=True, stop=True)

            # ---- InstanceNorm (rstd folded into W2) + ReLU + W2 + out ----
            hn_sb = reuse.tile([128, 4, N], BF16, name="hn", tag="big")
            out_sb = reuse.tile([128, 4, 2, CHUNK], F16, name="outsb", tag="big")
            nmean = small.tile([128, 4], F32, name="nmean", tag="mean")
            var4 = small.tile([128, 4], F32, name="var4", tag="var4")
            for o in range(4):
                mv = small.tile([128, 2], F32, name="mv", tag="mv")
                nc.vector.bn_aggr(mv[:], stats_sb[:, o, :, :])
                nc.vector.tensor_scalar_mul(nmean[:, o : o + 1], mv[:, 0:1], -1.0)
                nc.vector.tensor_copy(var4[:, o : o + 1], mv[:, 1:2])
            # rstd via DVE-only Newton rsqrt (no scalar table loads):
            # y0 = 1/(0.5 + 0.5 v), then 4x y <- y(1.5 - 0.5 v y^2)
            veps = small.tile([128, 4], F32, name="veps", tag="veps")
            nc.vector.tensor_scalar_add(veps[:], var4[:], EPS)
            v4e = small.tile([128, 4], F32, name="v4e", tag="v4e")
            nc.vector.tensor_scalar(v4e[:], veps[:], 0.5, 0.5,
                                    op0=ALU.mult, op1=ALU.add)
            rstd4 = small.tile([128, 4], F32, name="rstd4", tag="rstd4")
            nc.vector.reciprocal_approx_fast(rstd4[:], v4e[:])
            tn = small.tile([128, 4], F32, name="tn", tag="tn")
            # Newton runs on gpsimd (idle here) so the serial chain doesn't
            # interleave with the 16 relu ops on the DVE queue
            for _ in range(3):
                nc.gpsimd.tensor_mul(tn[:], rstd4[:], rstd4[:])
                nc.gpsimd.tensor_mul(tn[:], tn[:], veps[:])
                nc.gpsimd.tensor_scalar(tn[:], tn[:], -0.5, 1.5,
                                        op0=ALU.mult, op1=ALU.add)
                nc.gpsimd.tensor_mul(rstd4[:], rstd4[:], tn[:])
            for o in range(4):
                nc.vector.tensor_scalar_mul(
                    w2_sb[:, o, :], w2_sb[:, o, :], rstd4[:, o : o + 1]
                )
            for j in range(NCH):
                for o in range(4):
                    # all relus on DVE: the scalar queue must stay clear so
                    # the out-stage identities aren't stuck behind them
                    nc.vector.tensor_scalar(
                        hn_sb[:, o, ts(j, CHUNK)],
                        h1_sb[:, o, ts(j, CHUNK)],
                        nmean[:, o : o + 1],
                        0.0,
                        op0=ALU.add,
                        op1=ALU.max,
                    )
                for c in range(2):
                    op = ps_aux.tile([128, CHUNK], F32, name="ops", tag="aux")
                    for ki, kk in enumerate((3, 2, 1, 0)):
                        nc.tensor.matmul(
                            op[:],
                            w2_sb[:, kk, ts(c, 128)],
                            hn_sb[:, kk, ts(j, CHUNK)],
                            start=(ki == 0),
                            stop=(ki == 3),
                        )
                    nc.scalar.activation(
                        out_sb[:, j, c, :], op[:], AF.Identity,
                        bias=bias_sb[:, c, 2:3]
                    )
                nc.sync.dma_start(out=out_d[:, j, :, :], in_=out_sb[:, j, :, :])

    nc.compile()
    return nc


_NC = None


def _get_nc():
    global _NC
    if _NC is None:
        _NC = _build()
    return _NC


def _pmajor(a, k):
    # [k*128, cols] -> [128, k, cols] partition-major contiguous
    cols = a.shape[1]
    return np.ascontiguousarray(a.reshape(k, 128, cols).transpose(1, 0, 2))


def kernel(**inputs):
    x = np.asarray(inputs["x"], np.float32)
    source = np.asarray(inputs["source"], np.float32)
    Wq = np.asarray(inputs["Wq"], np.float32)
    bq = np.asarray(inputs["bq"], np.float32)
    Wk = np.asarray(inputs["Wk"], np.float32)
    bk = np.asarray(inputs["bk"], np.float32)
    Wv = np.asarray(inputs["Wv"], np.float32)
    bv = np.asarray(inputs["bv"], np.float32)
    Wm = np.asarray(inputs["Wm"], np.float64)
    W1 = np.asarray(inputs["W1"], np.float64)
    W2 = np.asarray(inputs["W2"], np.float32)
    b2 = np.asarray(inputs["b2"], np.float32)

    bf = ml_dtypes.bfloat16
    wqT = _pmajor(np.ascontiguousarray(Wq.reshape(H * DH, D).T), 2).astype(bf)
    wkT = _pmajor(np.ascontiguousarray(Wk.reshape(H * DH, D).T), 2).astype(bf)
    wvT = _pmajor(np.ascontiguousarray(Wv.reshape(H * DH, D).T), 2).astype(bf)
    WmP = Wm.reshape(D, DH, H).transpose(0, 2, 1).reshape(D, D)
    W1mWm = W1[:, D:] @ WmP
    w1T = _pmajor(
        np.vstack([W1[:, :D].T, W1mWm.T]).astype(np.float32), 4
    ).astype(bf)
    w2T = _pmajor(np.ascontiguousarray(W2.T), 4).astype(bf)
    bias = _pmajor(
        np.stack(
            [bq.reshape(D).astype(np.float32), bk.reshape(D).astype(np.float32),
             b2.reshape(D)], axis=1
        ),
        2,
    )
    shared = {
        "wqT": wqT,
        "wkT": wkT,
        "wvT": wvT,
        "w1T": np.ascontiguousarray(w1T),
        "w2T": w2T,
        "bias": np.ascontiguousarray(bias),
        "bv": np.ascontiguousarray(bv.reshape(1, D)).astype(bf),
    }
    in_maps = []
    for b in range(B):
        m = dict(shared)
        xp = _pmajor(x[b], 2).astype(bf)
        sp = _pmajor(source[b], 2).astype(bf)
        for j in range(4):
            m[f"x{j}"] = np.ascontiguousarray(xp[:, :, 512 * j : 512 * (j + 1)])
            m[f"src{j}"] = np.ascontiguousarray(sp[:, :, 512 * j : 512 * (j + 1)])
        in_maps.append(m)

    nc = _get_nc()
    try:
        res = run_bass_kernel_spmd(nc, in_maps, core_ids=list(range(B)))
    except Exception:
        res = run_bass_kernel_spmd(nc, in_maps, core_ids=list(range(B)))
    outs = []
    for b in range(B):
        arr = res.results[b]["out"].astype(np.float32)  # [128,4,2,512]
        outs.append(
            np.ascontiguousarray(arr.transpose(2, 0, 1, 3)).reshape(D, N)
        )
    return np.stack(outs, axis=0)


# revision 3
# speedup vs baseline: 1.1703x; 1.1703x over previous
"""AttentionalPropagation on 8 TRN2 NeuronCores — v3.

Data parallel over batch (B=8 -> one element per core). Math identical to v2
(bf16 matmuls f32 accum, Wm folded into W1, rstd folded into W2, exp without
max-subtraction). v3 changes the plumbing:

  - Inputs land partition-major and contiguous (one DMA packet per partition)
    and the dma_start instructions are spread across four engine queues so
    descriptor generation doesn't serialize the prologue.
  - The whole kernel is a sequence of 8 ACT-paced "windows" W(j, p) of 11
    score/exp groups (3x512 psum slots -> 1536-elem exp ACTs, 6-bank
    ping-pong). Aux tensor work (den duos, msg pairs, h1, QKV/vT projections)
    is emitted as generator segments, one per group, sized ~1.2us so the PE
    queue never buries the score matmuls the exp stream is waiting on.
  - scores: row-tiled K=64 pairs (heads on partitions 0-63/64-127 run
    concurrently); msg: col-tiled M=64 pairs; den: col-tiled M=1 duos.
  - Aux psum: 2 banks ping-pong (den/msg/h1/qk/vT/out chains are compact).

Window layout (den/msg of (j,p) consumed one window later):
  W(0,p0): scores + [k-p1, q-p1-j0, vT 0..9]
  W(0,p1): scores + [vT 10..15, den(0,p0), msg(0,p0), q j1..3]
  W(j,p0): scores + [den(j-1,p1), msg(j-1,p1), h1(j-1) o0,o1]
  W(j,p1): scores + [den(j,p0),   msg(j,p0),   h1(j-1) o2,o3]
  tail:    den(3,p1), msg(3,p1), h1(3), InstanceNorm, relu, W2, out
"""

import os
import sys

for _p in ("/opt/trn_rl_repo",):
    if _p not in sys.path:
        sys.path.insert(0, _p)

import numpy as np
import ml_dtypes

import concourse.bass as bass
import concourse.mybir as mybir
from concourse import bacc
from concourse import library_config
from concourse.bass import ts
from concourse.tile import TileContext
from concourse.bass_utils import run_bass_kernel_spmd

F32 = mybir.dt.float32
F16 = mybir.dt.float16
BF16 = mybir.dt.bfloat16
AF = mybir.ActivationFunctionType
ALU = mybir.AluOpType

B, D, N, M, H, DH = 8, 256, 2048, 2048, 4, 64
EPS = 1e-5
NCH = 4
CHUNK = 512
NMT = M // 128           # 16 m-tiles
PSLOTS = NMT * 2         # 32 slots per (j, p)
NGRP = (PSLOTS + 2) // 3  # 11 groups (10x3 + 1x2)


def _build():
    nc = bacc.Bacc("TRN2", target_bir_lowering=False, debug=False, num_devices=8)

    x_ds = [nc.dram_tensor(f"x{j}", [128, 2, CHUNK], BF16,
                           kind="ExternalInput").ap()
            for j in range(4)]
    s_ds = [nc.dram_tensor(f"src{j}", [128, 2, CHUNK], BF16,
                           kind="ExternalInput").ap()
            for j in range(4)]
    wq_d = nc.dram_tensor("wqT", [128, 2, D], BF16, kind="ExternalInput").ap()
    wk_d = nc.dram_tensor("wkT", [128, 2, D], BF16, kind="ExternalInput").ap()
    wv_d = nc.dram_tensor("wvT", [128, 2, D], BF16, kind="ExternalInput").ap()
    w1_d = nc.dram_tensor("w1T", [128, 4, 2 * D], BF16, kind="ExternalInput").ap()
    w2_d = nc.dram_tensor("w2T", [128, 4, D], BF16, kind="ExternalInput").ap()
    bias_d = nc.dram_tensor("bias", [128, 2, 3], F32, kind="ExternalInput").ap()
    bv_d = nc.dram_tensor("bv", [1, D], BF16, kind="ExternalInput").ap()
    out_d = nc.dram_tensor("out", [128, 4, 2, CHUNK], F16,
                           kind="ExternalOutput").ap()

    with TileContext(nc) as tc:
        nc.gpsimd.load_library(library_config.attn)
        with (
            tc.tile_pool(name="const", bufs=1) as const,
            tc.tile_pool(name="data", bufs=1) as data,
            tc.tile_pool(name="reuse", bufs=2) as reuse,
            tc.tile_pool(name="exps", bufs=3) as exps,
            tc.tile_pool(name="small", bufs=2) as small,
            tc.tile_pool(name="rbcs", bufs=2) as rbcs,
            tc.tile_pool(name="msgn", bufs=4) as msgn,
            tc.tile_pool(name="gate", bufs=1) as gatep,
            tc.tile_pool(name="ps_sc", bufs=2, space="PSUM") as ps_sc,
            tc.tile_pool(name="ps_aux", bufs=2, space="PSUM") as ps_aux,
        ):
            # ---- priority DMAs (contiguous per partition): the tensors that
            # gate the first K/Q projections ride sync/scalar immediately ----
            s_sb = reuse.tile([128, 2, M], BF16, name="s", tag="big")
            wk_sb = const.tile([128, 2, D], BF16, name="wk")
            x_sb = data.tile([128, 2, N], BF16, name="x")
            wq_sb = const.tile([128, 2, D], BF16, name="wq")
            wv_sb = const.tile([128, 2, D], BF16, name="wv")
            w1_sb = const.tile([128, 4, 2 * D], BF16, name="w1")
            w2_sb = const.tile([128, 4, D], BF16, name="w2")
            bias_sb = const.tile([128, 2, 3], F32, name="bias")
            bv_bc = const.tile([128, D], BF16, name="bvbc")

            nc.sync.dma_start(out=bias_sb[:], in_=bias_d)
            nc.sync.dma_start(out=s_sb[:, :, 0:CHUNK], in_=s_ds[0])
            nc.scalar.dma_start(out=wk_sb[:], in_=wk_d)
            nc.scalar.dma_start(out=wq_sb[:], in_=wq_d)
            nc.scalar.dma_start(out=wv_sb[:], in_=wv_d)
            nc.sync.dma_start(out=x_sb[:, :, 0:CHUNK], in_=x_ds[0])
            for jm in range(1, 4):
                nc.sync.dma_start(out=s_sb[:, :, ts(jm, CHUNK)], in_=s_ds[jm])

            eps_sb = const.tile([128, 1], F32, name="eps")
            nc.vector.memset(eps_sb[:], EPS)
            ones_sb = const.tile([128, 4], BF16, name="ones")
            nc.vector.memset(ones_sb[:], 1.0)
            dummy_sb = const.tile([128, 128], BF16, name="dummy")
            nc.vector.memset(dummy_sb[:], 0.0)
            for _ in range(6):
                wup = ps_aux.tile([128, 512], F32, name="wup", tag="aux")
                nc.tensor.matmul(wup[:, 0:128], dummy_sb[:], dummy_sb[:],
                                 start=True, stop=True)

            # ---- persistent SBUF ----
            q_sb = data.tile([128, 2, N], BF16, name="q")
            k_sb = data.tile([128, 2, M], BF16, name="k")
            vT_sb = [data.tile([128, H, DH], BF16, name=f"vT{t}")
                     for t in range(NMT)]
            h1_sb = data.tile([128, 4, N], BF16, name="h1")
            stats_sb = data.tile([128, 4, NCH, 6], F32, name="stats")

            eS = {}   # (j, p) -> [128, 32, 512] bf16
            mn = {}   # (j, p) -> [128, 512] bf16
            rbc = {}  # (j, h) -> [64, 512] f32

            def eS_view(j, p):
                return eS[(j, p)][:].rearrange("q (mt h) n -> q mt h n", h=2)

            # ---- aux emitters (generators yield ~1.2us segments) ----
            def qk_chunk(dst, w_sb, p, jm, b_col, src_t):
                ps = ps_aux.tile([128, CHUNK], F32, name="qk", tag="aux")
                for c in range(2):
                    nc.tensor.matmul(
                        ps[:],
                        w_sb[:, c, ts(p, 128)],
                        src_t[:, c, ts(jm, CHUNK)],
                        start=(c == 0),
                        stop=(c == 1),
                    )
                nc.vector.tensor_scalar_add(
                    dst[:, p, ts(jm, CHUNK)], ps[:], bias_sb[:, p, b_col : b_col + 1]
                )

            def vT_one(t):
                vp = ps_aux.tile([128, D], F32, name="vps", tag="aux")
                for c in range(2):
                    nc.tensor.matmul(
                        vp[:],
                        s_sb[:, c, ts(t, 128)],
                        wv_sb[:, c, :],
                        start=(c == 0),
                        stop=(c == 1),
                    )
                nc.vector.tensor_add(
                    vT_sb[t][:],
                    vp[:].rearrange("p (h d) -> p h d", h=H),
                    bv_bc[:].rearrange("p (h d) -> p h d", h=H),
                )

            def den_gen(j, p):
                dp = ps_aux.tile([128, CHUNK], F32, name="denps", tag="aux")
                v = eS_view(j, p)
                for mt in range(NMT):
                    if mt and mt % 4 == 0:
                        yield
                    for h2 in range(2):
                        h = 2 * p + h2
                        nc.tensor.matmul(
                            dp[32 * h : 32 * h + 1, :],
                            ones_sb[:, h : h + 1],
                            v[:, mt, h2, :],
                            start=(mt == 0),
                            stop=(mt == NMT - 1),
                            tile_position=(0, 32 * h),
                        )
                dens, rdens = [], []
                for h2 in range(2):
                    h = 2 * p + h2
                    den = small.tile([1, CHUNK], F32, name="den", tag="den")
                    nc.vector.tensor_copy(den[:], dp[32 * h : 32 * h + 1, :])
                    dens.append(den)
                for h2 in range(2):
                    rden = small.tile([1, CHUNK], F32, name="rden", tag="rden")
                    nc.vector.reciprocal_approx_fast(rden[:], dens[h2][:])
                    rdens.append(rden)
                for h2 in range(2):
                    rb = rbcs.tile([DH, CHUNK], F32, name="rbc", tag="rbc")
                    nc.gpsimd.partition_broadcast(rb[:], rdens[h2][:])
                    rbc[(j, 2 * p + h2)] = rb

            def msg_gen(j, p):
                mp = ps_aux.tile([128, CHUNK], F32, name="msgps", tag="aux")
                v = eS_view(j, p)
                for mt in range(NMT):
                    if mt and mt % 4 == 0:
                        yield
                    for h2 in range(2):
                        nc.tensor.matmul(
                            mp[ts(h2, DH), :],
                            vT_sb[mt][:, 2 * p + h2, :],
                            v[:, mt, h2, :],
                            start=(mt == 0),
                            stop=(mt == NMT - 1),
                        )
                mnp = msgn.tile([128, CHUNK], BF16, name="mn", tag="mn")
                for h2 in range(2):
                    nc.vector.tensor_mul(
                        mnp[ts(h2, DH), :],
                        mp[ts(h2, DH), :],
                        rbc.pop((j, 2 * p + h2))[:],
                    )
                mn[(j, p)] = mnp

            def h1_gen(j, olist):
                for o in olist:
                    hp = ps_aux.tile([128, CHUNK], F32, name="h1ps", tag="aux")
                    for c in range(4):
                        rhs = (
                            x_sb[:, c, ts(j, CHUNK)] if c < 2 else mn[(j, c - 2)][:]
                        )
                        nc.tensor.matmul(
                            hp[:],
                            w1_sb[:, c, ts(o, 128)],
                            rhs,
                            start=(c == 0),
                            stop=(c == 3),
                        )
                    nc.vector.tensor_copy(h1_sb[:, o, ts(j, CHUNK)], hp[:])
                    nc.vector.bn_stats(
                        stats_sb[:, o, j, :], h1_sb[:, o, ts(j, CHUNK)]
                    )
                    if o != olist[-1]:
                        yield

            def list_gen(thunks, per_seg):
                for i, t in enumerate(thunks):
                    if i and i % per_seg == 0:
                        yield
                    t()

            class Trail:
                """den duo + msg pair chains of (j, p) emitted mt-by-mt,
                lagging the exp stream by 2 ACT groups (used only for the
                final window so its softmax consumers finish with the exps).
                """

                def __init__(self, j, p):
                    self.j, self.p = j, p
                    self.mt = 0
                    self.dp = ps_aux.tile([128, CHUNK], F32, name="denps",
                                          tag="aux")
                    self.mp = ps_aux.tile([128, CHUNK], F32, name="msgps",
                                          tag="aux")
                    self.v = eS_view(j, p)

                def advance(self, mt_lim):
                    p = self.p
                    while self.mt <= min(mt_lim, NMT - 1):
                        mt = self.mt
                        for h2 in range(2):
                            h = 2 * p + h2
                            nc.tensor.matmul(
                                self.dp[32 * h : 32 * h + 1, :],
                                ones_sb[:, h : h + 1],
                                self.v[:, mt, h2, :],
                                start=(mt == 0),
                                stop=(mt == NMT - 1),
                                tile_position=(0, 32 * h),
                            )
                        for h2 in range(2):
                            nc.tensor.matmul(
                                self.mp[ts(h2, DH), :],
                                vT_sb[mt][:, 2 * p + h2, :],
                                self.v[:, mt, h2, :],
                                start=(mt == 0),
                                stop=(mt == NMT - 1),
                            )
                        self.mt += 1

                def finish(self):
                    self.advance(NMT - 1)
                    j, p = self.j, self.p
                    dens, rdens = [], []
                    for h2 in range(2):
                        h = 2 * p + h2
                        den = small.tile([1, CHUNK], F32, name="den", tag="den")
                        nc.vector.tensor_copy(
                            den[:], self.dp[32 * h : 32 * h + 1, :]
                        )
                        dens.append(den)
                    for h2 in range(2):
                        rden = small.tile([1, CHUNK], F32, name="rden",
                                          tag="rden")
                        nc.vector.reciprocal_approx_fast(rden[:], dens[h2][:])
                        rdens.append(rden)
                    for h2 in range(2):
                        rb = rbcs.tile([DH, CHUNK], F32, name="rbc", tag="rbc")
                        nc.gpsimd.partition_broadcast(rb[:], rdens[h2][:])
                        rbc[(j, 2 * p + h2)] = rb
                    mnp = msgn.tile([128, CHUNK], BF16, name="mn", tag="mn")
                    for h2 in range(2):
                        nc.vector.tensor_mul(
                            mnp[ts(h2, DH), :],
                            self.mp[ts(h2, DH), :],
                            rbc.pop((j, 2 * p + h2))[:],
                        )
                    mn[(j, p)] = mnp

            # ---- window: 11 ACT-paced score groups + one aux segment each ----
            def window(j, p, gens, trail=False):
                eS[(j, p)] = exps.tile(
                    [128, PSLOTS, CHUNK], BF16, name="eS", tag="eS"
                )
                gq = list(gens)
                tr = None
                for g in range(NGRP):
                    lo = 3 * g
                    nu = min(3, PSLOTS - lo)
                    sc = ps_sc.tile([128, 3, CHUNK], F32, name="sc", tag="sc")
                    for u in range(nu):
                        mt, h2 = divmod(lo + u, 2)
                        nc.tensor.matmul(
                            sc[:, u, :],
                            k_sb[ts(h2, DH), p, ts(mt, 128)],
                            q_sb[ts(h2, DH), p, ts(j, CHUNK)],
                            start=True,
                            stop=True,
                        )
                    nc.scalar.activation(
                        eS[(j, p)][:, lo : lo + nu, :],
                        sc[:, 0:nu, :],
                        AF.Exp,
                        scale=1.0 / 8.0,
                    )
                    if gq:
                        try:
                            next(gq[0])
                        except StopIteration:
                            gq.pop(0)
                    elif trail and g >= 2:
                        if tr is None:
                            tr = Trail(j, p)
                        tr.advance((3 * g - 5) // 2)
                while gq:
                    try:
                        next(gq[0])
                    except StopIteration:
                        gq.pop(0)
                return tr

            # ---- schedule ----
            # prologue: K proj chunk 0 gates scores(0, p0) group 0
            qk_chunk(k_sb, wk_sb, 0, 0, 1, s_sb)
            # deferred DMAs: WAW pre-touch via the gate value keeps the
            # scheduler from hoisting them into the priority transfer window
            gate_sb = gatep.tile([1, 4], BF16, name="gate")
            nc.gpsimd.tensor_copy(gate_sb[:], k_sb[0:1, 0, 0:4])
            deferred = [
                (x_sb[0:1, 0, CHUNK : CHUNK + 1],
                 x_sb[:, :, ts(1, CHUNK)], x_ds[1]),
                (x_sb[0:1, 0, 2 * CHUNK : 2 * CHUNK + 1],
                 x_sb[:, :, ts(2, CHUNK)], x_ds[2]),
                (x_sb[0:1, 0, 3 * CHUNK : 3 * CHUNK + 1],
                 x_sb[:, :, ts(3, CHUNK)], x_ds[3]),
                (w1_sb[0:1, 0, 0:1], w1_sb[:], w1_d),
                (w2_sb[0:1, 0, 0:1], w2_sb[:], w2_d),
            ]
            for touch, dst, src in deferred:
                nc.vector.tensor_copy(touch, gate_sb[0:1, 0:1])
            for touch, dst, src in deferred:
                nc.gpsimd.dma_start(out=dst, in_=src)
            bv_src = bass.AP(
                tensor=bv_d.tensor, offset=bv_d.offset, ap=[[0, 128]] + bv_d.ap[1:]
            )
            nc.vector.tensor_copy(bv_bc[0:1, 0:1], gate_sb[0:1, 0:1])
            nc.gpsimd.dma_start(out=bv_bc[:], in_=bv_src)
            qk_chunk(q_sb, wq_sb, 0, 0, 0, x_sb)
            for jm in range(1, 4):
                qk_chunk(k_sb, wk_sb, 0, jm, 1, s_sb)

            window(0, 0, [
                list_gen([lambda jm=jm: qk_chunk(k_sb, wk_sb, 1, jm, 1, s_sb)
                          for jm in range(4)]
                         + [lambda: qk_chunk(q_sb, wq_sb, 1, 0, 0, x_sb)], 2),
                list_gen([lambda t=t: vT_one(t) for t in range(10)], 3),
            ])
            window(0, 1, [
                list_gen([lambda t=t: vT_one(t) for t in range(10, 16)], 3),
                den_gen(0, 0),
                msg_gen(0, 0),
                list_gen([lambda p=p, jm=jm: qk_chunk(q_sb, wq_sb, p, jm, 0, x_sb)
                          for jm in range(1, 4) for p in range(2)], 2),
            ])
            for j in range(1, NCH - 1):
                window(j, 0, [
                    den_gen(j - 1, 1),
                    msg_gen(j - 1, 1),
                    h1_gen(j - 1, [0, 1]),
                ])
                window(j, 1, [
                    den_gen(j, 0),
                    msg_gen(j, 0),
                    h1_gen(j - 1, [2, 3]),
                ])
            # last chunk: W(3,p1) trails its own den/msg; h1(2) o2/o3 move to
            # the tail where the PE would otherwise idle during the mn chain
            window(3, 0, [
                den_gen(2, 1),
                msg_gen(2, 1),
                h1_gen(2, [0, 1]),
            ])
            tr = window(3, 1, [
                den_gen(3, 0),
            ], trail=True)

            jL = NCH - 1
            # msg(3,0) moves out of the overloaded last window into the tail:
            # its deps (eS(3,0), vT, rbc(3,0/1)) are all ready, so it runs
            # immediately and fills the PE idle while the mn chain resolves
            for _ in msg_gen(3, 0):
                pass
            if tr is None:
                tr = Trail(jL, 1)
            tr.finish()
            # PE work whose deps are long ready fills the mn-chain latency
            for _ in h1_gen(2, [2, 3]):
                pass
            for _ in range(10):
                wup2 = ps_sc.tile([128, 3, CHUNK], F32, name="wup2", tag="sc")
                nc.tensor.matmul(wup2[:, 0, 0:128], dummy_sb[:], dummy_sb[:],
                                 start=True, stop=True)
            for _ in h1_gen(jL, [0, 1, 2, 3]):
                pass
            for _ in range(26):
                wup3 = ps_sc.tile([128, 3, CHUNK], F32, name="wup3", tag="sc")
                nc.tensor.matmul(wup3[:, 0, 0:128], dummy_sb[:], dummy_sb[:],
                                 start=True, stop=True)

            # ---- InstanceNorm (rstd folded into W2) + ReLU + W2 + out ----
            hn_sb = reuse.tile([128, 4, N], BF16, name="hn", tag="big")
            out_sb = reuse.tile([128, 4, 2, CHUNK], F16, name="outsb", tag="big")
            nmean = small.tile([128, 4], F32, name="nmean", tag="mean")
            var4 = small.tile([128, 4], F32, name="var4", tag="var4")
            for o in range(4):
                mv = small.tile([128, 2], F32, name="mv", tag="mv")
                nc.vector.bn_aggr(mv[:], stats_sb[:, o, :, :])
                nc.vector.tensor_scalar_mul(nmean[:, o : o + 1], mv[:, 0:1], -1.0)
                nc.vector.tensor_copy(var4[:, o : o + 1], mv[:, 1:2])
            # rstd via DVE-only Newton rsqrt (no scalar table loads):
            # y0 = 1/(0.5 + 0.5 v), then 4x y <- y(1.5 - 0.5 v y^2)
            veps = small.tile([128, 4], F32, name="veps", tag="veps")
            nc.vector.tensor_scalar_add(veps[:], var4[:], EPS)
            v4e = small.tile([128, 4], F32, name="v4e", tag="v4e")
            nc.vector.tensor_scalar(v4e[:], veps[:], 0.5, 0.5,
                                    op0=ALU.mult, op1=ALU.add)
            rstd4 = small.tile([128, 4], F32, name="rstd4", tag="rstd4")
            nc.vector.reciprocal_approx_fast(rstd4[:], v4e[:])
            tn = small.tile([128, 4], F32, name="tn", tag="tn")
            # Newton runs on gpsimd (idle here) so the serial chain doesn't
            # interleave with the 16 relu ops on the DVE queue
            for _ in range(3):
                nc.gpsimd.tensor_mul(tn[:], rstd4[:], rstd4[:])
                nc.gpsimd.tensor_mul(tn[:], tn[:], veps[:])
                nc.gpsimd.tensor_scalar(tn[:], tn[:], -0.5, 1.5,
                                        op0=ALU.mult, op1=ALU.add)
                nc.gpsimd.tensor_mul(rstd4[:], rstd4[:], tn[:])
            for o in range(4):
                nc.vector.tensor_scalar_mul(
                    w2_sb[:, o, :], w2_sb[:, o, :], rstd4[:, o : o + 1]
                )
            for j in range(NCH):
                for o in range(4):
                    # all relus on DVE: the scalar queue must stay clear so
                    # the out-stage identities aren't stuck behind them
                    nc.vector.tensor_scalar(
                        hn_sb[:, o, ts(j, CHUNK)],
                        h1_sb[:, o, ts(j, CHUNK)],
                        nmean[:, o : o + 1],
                        0.0,
                        op0=ALU.add,
                        op1=ALU.max,
                    )
                for c in range(2):
                    op = ps_aux.tile([128, CHUNK], F32, name="ops", tag="aux")
                    for ki, kk in enumerate((3, 2, 1, 0)):
                        nc.tensor.matmul(
                            op[:],
                            w2_sb[:, kk, ts(c, 128)],
                            hn_sb[:, kk, ts(j, CHUNK)],
                            start=(ki == 0),
                            stop=(ki == 3),
                        )
                    nc.scalar.activation(
                        out_sb[:, j, c, :], op[:], AF.Identity,
                        bias=bias_sb[:, c, 2:3]
                    )
                nc.sync.dma_start(out=out_d[:, j, :, :], in_=out_sb[:, j, :, :])

    nc.compile()
    return nc


_NC = None


def _get_nc():
    global _NC
    if _NC is None:
        _NC = _build()
    return _NC


def _pmajor(a, k):
    # [k*128, cols] -> [128, k, cols] partition-major contiguous
    cols = a.shape[1]
    return np.ascontiguousarray(a.reshape(k, 128, cols).transpose(1, 0, 2))


def kernel(**inputs):
    x = np.asarray(inputs["x"], np.float32)
    source = np.asarray(inputs["source"], np.float32)
    Wq = np.asarray(inputs["Wq"], np.float32)
    bq = np.asarray(inputs["bq"], np.float32)
    Wk = np.asarray(inputs["Wk"], np.float32)
    bk = np.asarray(inputs["bk"], np.float32)
    Wv = np.asarray(inputs["Wv"], np.float32)
    bv = np.asarray(inputs["bv"], np.float32)
    Wm = np.asarray(inputs["Wm"], np.float64)
    W1 = np.asarray(inputs["W1"], np.float64)
    W2 = np.asarray(inputs["W2"], np.float32)
    b2 = np.asarray(inputs["b2"], np.float32)

    bf = ml_dtypes.bfloat16
    wqT = _pmajor(np.ascontiguousarray(Wq.reshape(H * DH, D).T), 2).astype(bf)
    wkT = _pmajor(np.ascontiguousarray(Wk.reshape(H * DH, D).T), 2).astype(bf)
    wvT = _pmajor(np.ascontiguousarray(Wv.reshape(H * DH, D).T), 2).astype(bf)
    WmP = Wm.reshape(D, DH, H).transpose(0, 2, 1).reshape(D, D)
    W1mWm = W1[:, D:] @ WmP
    w1T = _pmajor(
        np.vstack([W1[:, :D].T, W1mWm.T]).astype(np.float32), 4
    ).astype(bf)
    w2T = _pmajor(np.ascontiguousarray(W2.T), 4).astype(bf)
    bias = _pmajor(
        np.stack(
            [bq.reshape(D).astype(np.float32), bk.reshape(D).astype(np.float32),
             b2.reshape(D)], axis=1
        ),
        2,
    )
    shared = {
        "wqT": wqT,
        "wkT": wkT,
        "wvT": wvT,
        "w1T": np.ascontiguousarray(w1T),
        "w2T": w2T,
        "bias": np.ascontiguousarray(bias),
        "bv": np.ascontiguousarray(bv.reshape(1, D)).astype(bf),
    }
    in_maps = []
    for b in range(B):
        m = dict(shared)
        xp = _pmajor(x[b], 2).astype(bf)
        sp = _pmajor(source[b], 2).astype(bf)
        for j in range(4):
            m[f"x{j}"] = np.ascontiguousarray(xp[:, :, 512 * j : 512 * (j + 1)])
            m[f"src{j}"] = np.ascontiguousarray(sp[:, :, 512 * j : 512 * (j + 1)])
        in_maps.append(m)

    nc = _get_nc()
    try:
        res = run_bass_kernel_spmd(nc, in_maps, core_ids=list(range(B)))
    except Exception:
        res = run_bass_kernel_spmd(nc, in_maps, core_ids=list(range(B)))
    outs = []
    for b in range(B):
        arr = res.results[b]["out"].astype(np.float32)  # [128,4,2,512]
        outs.append(
            np.ascontiguousarray(arr.transpose(2, 0, 1, 3)).reshape(D, N)
        )
    return np.stack(outs, axis=0)


# revision 4
# speedup vs baseline: 1.1831x; 1.0109x over previous
"""AttentionalPropagation on 8 TRN2 NeuronCores — v3.

Data parallel over batch (B=8 -> one element per core). Math identical to v2
(bf16 matmuls f32 accum, Wm folded into W1, rstd folded into W2, exp without
max-subtraction). v3 changes the plumbing:

  - Inputs land partition-major and contiguous (one DMA packet per partition)
    and the dma_start instructions are spread across four engine queues so
    descriptor generation doesn't serialize the prologue.
  - The whole kernel is a sequence of 8 ACT-paced "windows" W(j, p) of 11
    score/exp groups (3x512 psum slots -> 1536-elem exp ACTs, 6-bank
    ping-pong). Aux tensor work (den duos, msg pairs, h1, QKV/vT projections)
    is emitted as generator segments, one per group, sized ~1.2us so the PE
    queue never buries the score matmuls the exp stream is waiting on.
  - scores: row-tiled K=64 pairs (heads on partitions 0-63/64-127 run
    concurrently); msg: col-tiled M=64 pairs; den: col-tiled M=1 duos.
  - Aux psum: 2 banks ping-pong (den/msg/h1/qk/vT/out chains are compact).

Window layout (den/msg of (j,p) consumed one window later):
  W(0,p0): scores + [k-p1, q-p1-j0, vT 0..9]
  W(0,p1): scores + [vT 10..15, den(0,p0), msg(0,p0), q j1..3]
  W(j,p0): scores + [den(j-1,p1), msg(j-1,p1), h1(j-1) o0,o1]
  W(j,p1): scores + [den(j,p0),   msg(j,p0),   h1(j-1) o2,o3]
  tail:    den(3,p1), msg(3,p1), h1(3), InstanceNorm, relu, W2, out
"""

import os
import sys

for _p in ("/opt/trn_rl_repo",):
    if _p not in sys.path:
        sys.path.insert(0, _p)

import numpy as np
import ml_dtypes

import concourse.bass as bass
import concourse.mybir as mybir
from concourse import bacc
from concourse import library_config
from concourse.bass import ts
from concourse.tile import TileContext
from concourse.bass_utils import run_bass_kernel_spmd

F32 = mybir.dt.float32
F16 = mybir.dt.float16
BF16 = mybir.dt.bfloat16
AF = mybir.ActivationFunctionType
ALU = mybir.AluOpType

B, D, N, M, H, DH = 8, 256, 2048, 2048, 4, 64
EPS = 1e-5
NCH = 4
CHUNK = 512
NMT = M // 128           # 16 m-tiles
PSLOTS = NMT * 2         # 32 slots per (j, p)
NGRP = (PSLOTS + 2) // 3  # 11 groups (10x3 + 1x2)


def _build():
    nc = bacc.Bacc("TRN2", target_bir_lowering=False, debug=False, num_devices=8)

    x_ds = [nc.dram_tensor(f"x{j}", [128, 2, CHUNK], BF16,
                           kind="ExternalInput").ap()
            for j in range(4)]
    s_ds = [nc.dram_tensor(f"src{j}", [128, 2, CHUNK], BF16,
                           kind="ExternalInput").ap()
            for j in range(4)]
    wq_d = nc.dram_tensor("wqT", [128, 2, D], BF16, kind="ExternalInput").ap()
    wk_d = nc.dram_tensor("wkT", [128, 2, D], BF16, kind="ExternalInput").ap()
    wv_d = nc.dram_tensor("wvT", [128, 2, D], BF16, kind="ExternalInput").ap()
    w1_d = nc.dram_tensor("w1T", [128, 4, 2 * D], BF16, kind="ExternalInput").ap()
    w2_d = nc.dram_tensor("w2T", [128, 4, D], BF16, kind="ExternalInput").ap()
    bias_d = nc.dram_tensor("bias", [128, 2, 3], F32, kind="ExternalInput").ap()
    bv_d = nc.dram_tensor("bv", [1, D], BF16, kind="ExternalInput").ap()
    out_d = nc.dram_tensor("out", [128, 4, 2, CHUNK], F16,
                           kind="ExternalOutput").ap()

    with TileContext(nc) as tc:
        nc.gpsimd.load_library(library_config.attn)
        with (
            tc.tile_pool(name="const", bufs=1) as const,
            tc.tile_pool(name="data", bufs=1) as data,
            tc.tile_pool(name="reuse", bufs=2) as reuse,
            tc.tile_pool(name="exps", bufs=3) as exps,
            tc.tile_pool(name="small", bufs=2) as small,
            tc.tile_pool(name="rbcs", bufs=2) as rbcs,
            tc.tile_pool(name="msgn", bufs=4) as msgn,
            tc.tile_pool(name="gate", bufs=1) as gatep,
            tc.tile_pool(name="ps_sc", bufs=2, space="PSUM") as ps_sc,
            tc.tile_pool(name="ps_aux", bufs=2, space="PSUM") as ps_aux,
        ):
            # ---- priority DMAs (contiguous per partition): the tensors that
            # gate the first K/Q projections ride sync/scalar immediately ----
            s_sb = reuse.tile([128, 2, M], BF16, name="s", tag="big")
            wk_sb = const.tile([128, 2, D], BF16, name="wk")
            x_sb = data.tile([128, 2, N], BF16, name="x")
            wq_sb = const.tile([128, 2, D], BF16, name="wq")
            wv_sb = const.tile([128, 2, D], BF16, name="wv")
            w1_sb = const.tile([128, 4, 2 * D], BF16, name="w1")
            w2_sb = const.tile([128, 4, D], BF16, name="w2")
            bias_sb = const.tile([128, 2, 3], F32, name="bias")
            bv_bc = const.tile([128, D], BF16, name="bvbc")

            nc.sync.dma_start(out=bias_sb[:], in_=bias_d)
            nc.sync.dma_start(out=s_sb[:, :, 0:CHUNK], in_=s_ds[0])
            nc.scalar.dma_start(out=wk_sb[:], in_=wk_d)
            nc.scalar.dma_start(out=wq_sb[:], in_=wq_d)
            nc.scalar.dma_start(out=wv_sb[:], in_=wv_d)
            nc.sync.dma_start(out=x_sb[:, :, 0:CHUNK], in_=x_ds[0])
            for jm in range(1, 4):
                nc.sync.dma_start(out=s_sb[:, :, ts(jm, CHUNK)], in_=s_ds[jm])

            eps_sb = const.tile([128, 1], F32, name="eps")
            nc.vector.memset(eps_sb[:], EPS)
            ones_sb = const.tile([128, 4], BF16, name="ones")
            nc.vector.memset(ones_sb[:], 1.0)
            dummy_sb = const.tile([128, 128], BF16, name="dummy")
            nc.vector.memset(dummy_sb[:], 0.0)
            for _ in range(6):
                wup = ps_aux.tile([128, 512], F32, name="wup", tag="aux")
                nc.tensor.matmul(wup[:, 0:128], dummy_sb[:], dummy_sb[:],
                                 start=True, stop=True)

            # ---- persistent SBUF ----
            q_sb = data.tile([128, 2, N], BF16, name="q")
            k_sb = data.tile([128, 2, M], BF16, name="k")
            vT_sb = [data.tile([128, H, DH], BF16, name=f"vT{t}")
                     for t in range(NMT)]
            h1_sb = data.tile([128, 4, N], BF16, name="h1")
            stats_sb = data.tile([128, 4, NCH, 6], F32, name="stats")

            eS = {}   # (j, p) -> [128, 32, 512] bf16
            mn = {}   # (j, p) -> [128, 512] bf16
            rbc = {}  # (j, h) -> [64, 512] f32

            def eS_view(j, p):
                return eS[(j, p)][:].rearrange("q (mt h) n -> q mt h n", h=2)

            # ---- aux emitters (generators yield ~1.2us segments) ----
            def qk_chunk(dst, w_sb, p, jm, b_col, src_t):
                ps = ps_aux.tile([128, CHUNK], F32, name="qk", tag="aux")
                for c in range(2):
                    nc.tensor.matmul(
                        ps[:],
                        w_sb[:, c, ts(p, 128)],
                        src_t[:, c, ts(jm, CHUNK)],
                        start=(c == 0),
                        stop=(c == 1),
                    )
                nc.vector.tensor_scalar_add(
                    dst[:, p, ts(jm, CHUNK)], ps[:], bias_sb[:, p, b_col : b_col + 1]
                )

            def vT_one(t):
                vp = ps_aux.tile([128, D], F32, name="vps", tag="aux")
                for c in range(2):
                    nc.tensor.matmul(
                        vp[:],
                        s_sb[:, c, ts(t, 128)],
                        wv_sb[:, c, :],
                        start=(c == 0),
                        stop=(c == 1),
                    )
                nc.vector.tensor_add(
                    vT_sb[t][:],
                    vp[:].rearrange("p (h d) -> p h d", h=H),
                    bv_bc[:].rearrange("p (h d) -> p h d", h=H),
                )

            def den_gen(j, p):
                dp = ps_aux.tile([128, CHUNK], F32, name="denps", tag="aux")
                v = eS_view(j, p)
                for mt in range(NMT):
                    if mt and mt % 4 == 0:
                        yield
                    for h2 in range(2):
                        h = 2 * p + h2
                        nc.tensor.matmul(
                            dp[32 * h : 32 * h + 1, :],
                            ones_sb[:, h : h + 1],
                            v[:, mt, h2, :],
                            start=(mt == 0),
                            stop=(mt == NMT - 1),
                            tile_position=(0, 32 * h),
                        )
                dens, rdens = [], []
                for h2 in range(2):
                    h = 2 * p + h2
                    den = small.tile([1, CHUNK], F32, name="den", tag="den")
                    nc.vector.tensor_copy(den[:], dp[32 * h : 32 * h + 1, :])
                    dens.append(den)
                for h2 in range(2):
                    rden = small.tile([1, CHUNK], F32, name="rden", tag="rden")
                    nc.vector.reciprocal_approx_fast(rden[:], dens[h2][:])
                    rdens.append(rden)
                for h2 in range(2):
                    rb = rbcs.tile([DH, CHUNK], F32, name="rbc", tag="rbc")
                    nc.gpsimd.partition_broadcast(rb[:], rdens[h2][:])
                    rbc[(j, 2 * p + h2)] = rb

            def msg_gen(j, p):
                mp = ps_aux.tile([128, CHUNK], F32, name="msgps", tag="aux")
                v = eS_view(j, p)
                for mt in range(NMT):
                    if mt and mt % 4 == 0:
                        yield
                    for h2 in range(2):
                        nc.tensor.matmul(
                            mp[ts(h2, DH), :],
                            vT_sb[mt][:, 2 * p + h2, :],
                            v[:, mt, h2, :],
                            start=(mt == 0),
                            stop=(mt == NMT - 1),
                        )
                mnp = msgn.tile([128, CHUNK], BF16, name="mn", tag="mn")
                for h2 in range(2):
                    nc.vector.tensor_mul(
                        mnp[ts(h2, DH), :],
                        mp[ts(h2, DH), :],
                        rbc.pop((j, 2 * p + h2))[:],
                    )
                mn[(j, p)] = mnp

            def h1_gen(j, olist):
                for o in olist:
                    hp = ps_aux.tile([128, CHUNK], F32, name="h1ps", tag="aux")
                    for c in range(4):
                        rhs = (
                            x_sb[:, c, ts(j, CHUNK)] if c < 2 else mn[(j, c - 2)][:]
                        )
                        nc.tensor.matmul(
                            hp[:],
                            w1_sb[:, c, ts(o, 128)],
                            rhs,
                            start=(c == 0),
                            stop=(c == 3),
                        )
                    nc.vector.tensor_copy(h1_sb[:, o, ts(j, CHUNK)], hp[:])
                    nc.vector.bn_stats(
                        stats_sb[:, o, j, :], h1_sb[:, o, ts(j, CHUNK)]
                    )
                    if o != olist[-1]:
                        yield

            def list_gen(thunks, per_seg):
                for i, t in enumerate(thunks):
                    if i and i % per_seg == 0:
                        yield
                    t()

            class Trail:
                """den duo + msg pair chains of (j, p) emitted mt-by-mt,
                lagging the exp stream by 2 ACT groups (used only for the
                final window so its softmax consumers finish with the exps).
                """

                def __init__(self, j, p):
                    self.j, self.p = j, p
                    self.mt = 0
                    self.dp = ps_aux.tile([128, CHUNK], F32, name="denps",
                                          tag="aux")
                    self.mp = ps_aux.tile([128, CHUNK], F32, name="msgps",
                                          tag="aux")
                    self.v = eS_view(j, p)

                def advance(self, mt_lim):
                    p = self.p
                    while self.mt <= min(mt_lim, NMT - 1):
                        mt = self.mt
                        for h2 in range(2):
                            h = 2 * p + h2
                            nc.tensor.matmul(
                                self.dp[32 * h : 32 * h + 1, :],
                                ones_sb[:, h : h + 1],
                                self.v[:, mt, h2, :],
                                start=(mt == 0),
                                stop=(mt == NMT - 1),
                                tile_position=(0, 32 * h),
                            )
                        for h2 in range(2):
                            nc.tensor.matmul(
                                self.mp[ts(h2, DH), :],
                                vT_sb[mt][:, 2 * p + h2, :],
                                self.v[:, mt, h2, :],
                                start=(mt == 0),
                                stop=(mt == NMT - 1),
                            )
                        self.mt += 1

                def finish(self):
                    self.advance(NMT - 1)
                    j, p = self.j, self.p
                    dens, rdens = [], []
                    for h2 in range(2):
                        h = 2 * p + h2
                        den = small.tile([1, CHUNK], F32, name="den", tag="den")
                        nc.vector.tensor_copy(
                            den[:], self.dp[32 * h : 32 * h + 1, :]
                        )
                        dens.append(den)
                    for h2 in range(2):
                        rden = small.tile([1, CHUNK], F32, name="rden",
                                          tag="rden")
                        nc.vector.reciprocal_approx_fast(rden[:], dens[h2][:])
                        rdens.append(rden)
                    for h2 in range(2):
                        rb = rbcs.tile([DH, CHUNK], F32, name="rbc", tag="rbc")
                        nc.gpsimd.partition_broadcast(rb[:], rdens[h2][:])
                        rbc[(j, 2 * p + h2)] = rb
                    mnp = msgn.tile([128, CHUNK], BF16, name="mn", tag="mn")
                    for h2 in range(2):
                        nc.vector.tensor_mul(
                            mnp[ts(h2, DH), :],
                            self.mp[ts(h2, DH), :],
                            rbc.pop((j, 2 * p + h2))[:],
                        )
                    mn[(j, p)] = mnp

            # ---- window: 11 ACT-paced score groups + one aux segment each ----
            def window(j, p, gens, trail=False):
                eS[(j, p)] = exps.tile(
                    [128, PSLOTS, CHUNK], BF16, name="eS", tag="eS"
                )
                gq = list(gens)
                tr = None
                for g in range(NGRP):
                    lo = 3 * g
                    nu = min(3, PSLOTS - lo)
                    sc = ps_sc.tile([128, 3, CHUNK], F32, name="sc", tag="sc")
                    for u in range(nu):
                        mt, h2 = divmod(lo + u, 2)
                        nc.tensor.matmul(
                            sc[:, u, :],
                            k_sb[ts(h2, DH), p, ts(mt, 128)],
                            q_sb[ts(h2, DH), p, ts(j, CHUNK)],
                            start=True,
                            stop=True,
                        )
                    nc.scalar.activation(
                        eS[(j, p)][:, lo : lo + nu, :],
                        sc[:, 0:nu, :],
                        AF.Exp,
                        scale=1.0 / 8.0,
                    )
                    if gq:
                        try:
                            next(gq[0])
                        except StopIteration:
                            gq.pop(0)
                    elif trail and g >= 2:
                        if tr is None:
                            tr = Trail(j, p)
                        tr.advance((3 * g - 5) // 2)
                while gq:
                    try:
                        next(gq[0])
                    except StopIteration:
                        gq.pop(0)
                return tr

            # ---- schedule ----
            # prologue: K proj chunk 0 gates scores(0, p0) group 0
            qk_chunk(k_sb, wk_sb, 0, 0, 1, s_sb)
            # deferred DMAs: WAW pre-touch via the gate value keeps the
            # scheduler from hoisting them into the priority transfer window
            gate_sb = gatep.tile([1, 4], BF16, name="gate")
            nc.gpsimd.tensor_copy(gate_sb[:], k_sb[0:1, 0, 0:4])
            deferred = [
                (x_sb[0:1, 0, CHUNK : CHUNK + 1],
                 x_sb[:, :, ts(1, CHUNK)], x_ds[1]),
                (x_sb[0:1, 0, 2 * CHUNK : 2 * CHUNK + 1],
                 x_sb[:, :, ts(2, CHUNK)], x_ds[2]),
                (x_sb[0:1, 0, 3 * CHUNK : 3 * CHUNK + 1],
                 x_sb[:, :, ts(3, CHUNK)], x_ds[3]),
                (w1_sb[0:1, 0, 0:1], w1_sb[:], w1_d),
                (w2_sb[0:1, 0, 0:1], w2_sb[:], w2_d),
            ]
            for touch, dst, src in deferred:
                nc.vector.tensor_copy(touch, gate_sb[0:1, 0:1])
            for touch, dst, src in deferred:
                nc.gpsimd.dma_start(out=dst, in_=src)
            bv_src = bass.AP(
                tensor=bv_d.tensor, offset=bv_d.offset, ap=[[0, 128]] + bv_d.ap[1:]
            )
            nc.vector.tensor_copy(bv_bc[0:1, 0:1], gate_sb[0:1, 0:1])
            nc.gpsimd.dma_start(out=bv_bc[:], in_=bv_src)
            qk_chunk(q_sb, wq_sb, 0, 0, 0, x_sb)
            for jm in range(1, 4):
                qk_chunk(k_sb, wk_sb, 0, jm, 1, s_sb)

            window(0, 0, [
                list_gen([lambda jm=jm: qk_chunk(k_sb, wk_sb, 1, jm, 1, s_sb)
                          for jm in range(4)]
                         + [lambda: qk_chunk(q_sb, wq_sb, 1, 0, 0, x_sb)], 2),
                list_gen([lambda t=t: vT_one(t) for t in range(16)], 3),
            ])
            window(0, 1, [
                den_gen(0, 0),
                msg_gen(0, 0),
                list_gen([lambda p=p, jm=jm: qk_chunk(q_sb, wq_sb, p, jm, 0, x_sb)
                          for jm in range(1, 4) for p in range(2)], 2),
            ])
            for j in range(1, NCH - 1):
                window(j, 0, [
                    den_gen(j - 1, 1),
                    msg_gen(j - 1, 1),
                    h1_gen(j - 1, [0, 1]),
                ])
                window(j, 1, [
                    den_gen(j, 0),
                    msg_gen(j, 0),
                    h1_gen(j - 1, [2, 3]),
                ])
            # last chunk: W(3,p1) trails its own den/msg; h1(2) o2/o3 move to
            # the tail where the PE would otherwise idle during the mn chain
            window(3, 0, [
                den_gen(2, 1),
                msg_gen(2, 1),
                h1_gen(2, [0, 1]),
            ])
            tr = window(3, 1, [
                den_gen(3, 0),
            ], trail=True)

            jL = NCH - 1
            # msg(3,0) moves out of the overloaded last window into the tail:
            # its deps (eS(3,0), vT, rbc(3,0/1)) are all ready, so it runs
            # immediately and fills the PE idle while the mn chain resolves
            for _ in msg_gen(3, 0):
                pass
            if tr is None:
                tr = Trail(jL, 1)
            tr.finish()
            # PE work whose deps are long ready fills the mn-chain latency
            for _ in h1_gen(2, [2, 3]):
                pass
            for _ in range(10):
                wup2 = ps_sc.tile([128, 3, CHUNK], F32, name="wup2", tag="sc")
                nc.tensor.matmul(wup2[:, 0, 0:128], dummy_sb[:], dummy_sb[:],
                                 start=True, stop=True)
            for _ in h1_gen(jL, [0, 1, 2, 3]):
                pass
            for _ in range(26):
                wup3 = ps_sc.tile([128, 3, CHUNK], F32, name="wup3", tag="sc")
                nc.tensor.matmul(wup3[:, 0, 0:128], dummy_sb[:], dummy_sb[:],
                                 start=True, stop=True)

            # ---- InstanceNorm (rstd folded into W2) + ReLU + W2 + out ----
            hn_sb = reuse.tile([128, 4, N], BF16, name="hn", tag="big")
            out_sb = reuse.tile([128, 4, 2, CHUNK], F16, name="outsb", tag="big")
            nmean = small.tile([128, 4], F32, name="nmean", tag="mean")
            var4 = small.tile([128, 4], F32, name="var4", tag="var4")
            for o in range(4):
                mv = small.tile([128, 2], F32, name="mv", tag="mv")
                nc.vector.bn_aggr(mv[:], stats_sb[:, o, :, :])
                nc.vector.tensor_scalar_mul(nmean[:, o : o + 1], mv[:, 0:1], -1.0)
                nc.vector.tensor_copy(var4[:, o : o + 1], mv[:, 1:2])
            # rstd via DVE-only Newton rsqrt (no scalar table loads):
            # y0 = 1/(0.5 + 0.5 v), then 4x y <- y(1.5 - 0.5 v y^2)
            veps = small.tile([128, 4], F32, name="veps", tag="veps")
            nc.vector.tensor_scalar_add(veps[:], var4[:], EPS)
            v4e = small.tile([128, 4], F32, name="v4e", tag="v4e")
            nc.vector.tensor_scalar(v4e[:], veps[:], 0.5, 0.5,
                                    op0=ALU.mult, op1=ALU.add)
            rstd4 = small.tile([128, 4], F32, name="rstd4", tag="rstd4")
            nc.vector.reciprocal_approx_fast(rstd4[:], v4e[:])
            tn = small.tile([128, 4], F32, name="tn", tag="tn")
            # Newton runs on gpsimd (idle here) so the serial chain doesn't
            # interleave with the 16 relu ops on the DVE queue
            for _ in range(3):
                nc.gpsimd.tensor_mul(tn[:], rstd4[:], rstd4[:])
                nc.gpsimd.tensor_mul(tn[:], tn[:], veps[:])
                nc.gpsimd.tensor_scalar(tn[:], tn[:], -0.5, 1.5,
                                        op0=ALU.mult, op1=ALU.add)
                nc.gpsimd.tensor_mul(rstd4[:], rstd4[:], tn[:])
            for o in range(4):
                nc.vector.tensor_scalar_mul(
                    w2_sb[:, o, :], w2_sb[:, o, :], rstd4[:, o : o + 1]
                )
            for j in range(NCH):
                for o in range(4):
                    # all relus on DVE: the scalar queue must stay clear so
                    # the out-stage identities aren't stuck behind them
                    nc.vector.tensor_scalar(
                        hn_sb[:, o, ts(j, CHUNK)],
                        h1_sb[:, o, ts(j, CHUNK)],
                        nmean[:, o : o + 1],
                        0.0,
                        op0=ALU.add,
                        op1=ALU.max,
                    )
                for c in range(2):
                    op = ps_aux.tile([128, CHUNK], F32, name="ops", tag="aux")
                    for ki, kk in enumerate((3, 2, 1, 0)):
                        nc.tensor.matmul(
                            op[:],
                            w2_sb[:, kk, ts(c, 128)],
                            hn_sb[:, kk, ts(j, CHUNK)],
                            start=(ki == 0),
                            stop=(ki == 3),
                        )
                    nc.scalar.activation(
                        out_sb[:, j, c, :], op[:], AF.Identity,
                        bias=bias_sb[:, c, 2:3]
                    )
                nc.sync.dma_start(out=out_d[:, j, :, :], in_=out_sb[:, j, :, :])

    nc.compile()
    return nc


_NC = None


def _get_nc():
    global _NC
    if _NC is None:
        _NC = _build()
    return _NC


def _pmajor(a, k):
    # [k*128, cols] -> [128, k, cols] partition-major contiguous
    cols = a.shape[1]
    return np.ascontiguousarray(a.reshape(k, 128, cols).transpose(1, 0, 2))


def kernel(**inputs):
    x = np.asarray(inputs["x"], np.float32)
    source = np.asarray(inputs["source"], np.float32)
    Wq = np.asarray(inputs["Wq"], np.float32)
    bq = np.asarray(inputs["bq"], np.float32)
    Wk = np.asarray(inputs["Wk"], np.float32)
    bk = np.asarray(inputs["bk"], np.float32)
    Wv = np.asarray(inputs["Wv"], np.float32)
    bv = np.asarray(inputs["bv"], np.float32)
    Wm = np.asarray(inputs["Wm"], np.float64)
    W1 = np.asarray(inputs["W1"], np.float64)
    W2 = np.asarray(inputs["W2"], np.float32)
    b2 = np.asarray(inputs["b2"], np.float32)

    bf = ml_dtypes.bfloat16
    wqT = _pmajor(np.ascontiguousarray(Wq.reshape(H * DH, D).T), 2).astype(bf)
    wkT = _pmajor(np.ascontiguousarray(Wk.reshape(H * DH, D).T), 2).astype(bf)
    wvT = _pmajor(np.ascontiguousarray(Wv.reshape(H * DH, D).T), 2).astype(bf)
    WmP = Wm.reshape(D, DH, H).transpose(0, 2, 1).reshape(D, D)
    W1mWm = W1[:, D:] @ WmP
    w1T = _pmajor(
        np.vstack([W1[:, :D].T, W1mWm.T]).astype(np.float32), 4
    ).astype(bf)
    w2T = _pmajor(np.ascontiguousarray(W2.T), 4).astype(bf)
    bias = _pmajor(
        np.stack(
            [bq.reshape(D).astype(np.float32), bk.reshape(D).astype(np.float32),
             b2.reshape(D)], axis=1
        ),
        2,
    )
    shared = {
        "wqT": wqT,
        "wkT": wkT,
        "wvT": wvT,
        "w1T": np.ascontiguousarray(w1T),
        "w2T": w2T,
        "bias": np.ascontiguousarray(bias),
        "bv": np.ascontiguousarray(bv.reshape(1, D)).astype(bf),
    }
    in_maps = []
    for b in range(B):
        m = dict(shared)
        xp = _pmajor(x[b], 2).astype(bf)
        sp = _pmajor(source[b], 2).astype(bf)
        for j in range(4):
            m[f"x{j}"] = np.ascontiguousarray(xp[:, :, 512 * j : 512 * (j + 1)])
            m[f"src{j}"] = np.ascontiguousarray(sp[:, :, 512 * j : 512 * (j + 1)])
        in_maps.append(m)

    nc = _get_nc()
    try:
        res = run_bass_kernel_spmd(nc, in_maps, core_ids=list(range(B)))
    except Exception:
        res = run_bass_kernel_spmd(nc, in_maps, core_ids=list(range(B)))
    outs = []
    for b in range(B):
        arr = res.results[b]["out"].astype(np.float32)  # [128,4,2,512]
        outs.append(
            np.ascontiguousarray(arr.transpose(2, 0, 1, 3)).reshape(D, N)
        )
    return np.stack(outs, axis=0)
